# revision 18
# baseline (speedup 1.0000x reference)
"""Causal multi-head self-attention on 8 TRN2 NeuronCores (Bass/Tile).

Problem: z[B=2,T=2048,D=1024], per-head dim 64, H=16 heads, fp32.
Sharding: core = b*4 + g  (b = batch, g = head-group of 4 heads).
Each core computes, for its batch b and heads 4g..4g+3:
    Q.T/K.T = (Wq/Wk slice).T-projection of z.T   [256, 2048] (head-stacked)
    V       = z @ Wv slice                         [2048, 256] (+ ones column)
    S.T     = K.T-slices vs Q.T  (partition = key j, free = query i)
    P       = exp(S/8) * causal mask   (no max-subtraction needed: |S|≲8σ)
    ctx.T   = [V | 1].T @ P   -> row 64 holds the softmax denominators
    out.T  += Wo-rows.T @ (ctx.T / denom)          [1024, 2048] partial
Host sums the 4 per-batch partials and transposes back.

All matmuls run as float32r (full-rate fp32 path; measured ~1.4e-4 relmax
vs fp64 on this HW, identical to plain fp32 output in the probe).
"""
import sys
import types

import numpy as np

# ── antenv.axon_hooks shim (NTFF profiling; agent image lacks the module) ──
import antenv  # noqa: F401

if "antenv.axon_hooks" not in sys.modules:
    _hooks = types.ModuleType("antenv.axon_hooks")
    _HOOK = [None]
    _hooks.set_axon_ntff_profile_hook = lambda h: _HOOK.__setitem__(0, h)
    _hooks.get_axon_ntff_profile_hook = lambda: _HOOK[0]
    sys.modules["antenv.axon_hooks"] = _hooks
    antenv.axon_hooks = _hooks
    try:
        from trn_agent_boot.trn_boot import _ntff_profile_via_ctypes

        _hooks.set_axon_ntff_profile_hook(
            _ntff_profile_via_ctypes("/opt/axon/libaxon_pjrt.so")
        )
    except Exception:
        pass

import concourse.bass as bass  # noqa: E402
import concourse.tile as tile  # noqa: E402
import concourse.mybir as mybir  # noqa: E402
import concourse.bass_utils as bass_utils  # noqa: E402
from bass_rust import ScopedClock  # noqa: E402

bass_utils.upload_artifacts = lambda tmpdir: ""

F32 = mybir.dt.float32
F32R = mybir.dt.float32r
EXP = mybir.ActivationFunctionType.Exp
LN = mybir.ActivationFunctionType.Ln

# ── workaround: this walrus build allows max ONE sync-wait per instruction ──
_wsplit = [0]


def _split_excess_waits(nc, limit=1):
    n = 0
    for fn in nc.m.functions:
        for blk in fn.blocks:
            out = []
            for inst in blk.instructions:
                si = inst.sync_info
                if si is not None and len(si.on_wait) > limit:
                    ws = list(si.on_wait)
                    keep = ws[-limit:]
                    hoist = ws[:-limit]
                    for i in range(0, len(hoist), limit):
                        _wsplit[0] += 1
                        out.append(
                            mybir.InstNoOp(
                                name=f"I-wsplit-{_wsplit[0]}",
                                engine=inst.engine,
                                sync_info=mybir.SyncInfo(
                                    on_wait=hoist[i : i + limit], on_update=[]
                                ),
                                bass_nofuse=True,
                            )
                        )
                        n += 1
                    si.on_wait = keep
                out.append(inst)
            blk.instructions = out
    return n


def _patched_drain_and_barrier(self, tick_clock, wait_clock):
    nc = self.nc
    drain_inst = nc.sync.drain()
    wait_clock.add_sem_waits(
        drain_inst.ins, ScopedClock({None: tick_clock.global_clock})
    )
    si = drain_inst.ins.sync_info
    if si is not None and len(si.on_wait) > 1:
        waits = list(si.on_wait)
        si.on_wait = waits[:1]
        for w in waits[1:]:
            d2 = nc.sync.drain()
            d2.ins.sync_info = mybir.SyncInfo(on_wait=[w], on_update=[])
    nc.all_engine_barrier()
    assert self.sems is not None
    popped = nc._tile_sem_poison_stack.pop()
    assert popped is self._sem_poison
    nc.clear_and_free_semaphores(list(self.sems.allocated().values()))
    nc.all_engine_barrier()


tile.TileContext._drain_and_barrier = _patched_drain_and_barrier

# ── problem shape (hardcoded) ──
B, T, D, H, HD = 2, 2048, 1024, 16, 64
HPC = 4  # heads per core
DG = HPC * HD  # 256 projection cols per core
NQ = 512  # query-chunk width (one PSUM bank of fp32)
KT = T // 128  # 16 key tiles
NCH = T // NQ  # 4 query chunks
D8 = D // 128  # 8 contraction tiles
SCALE = 1.0 / np.sqrt(HD)


def build_kernel():
    nc = bass.Bass("TRN2", target_bir_lowering=False, debug=False)
    zt_d = nc.dram_tensor("zt", [D, T], F32R, kind="ExternalInput").ap()
    wq_d = nc.dram_tensor("wq", [D, DG], F32R, kind="ExternalInput").ap()
    wk_d = nc.dram_tensor("wk", [D, DG], F32R, kind="ExternalInput").ap()
    wv_d = nc.dram_tensor("wv", [D, DG], F32R, kind="ExternalInput").ap()
    wo_d = nc.dram_tensor("wo", [DG, D], F32R, kind="ExternalInput").ap()
    mk_d = nc.dram_tensor("mk", [128, 128], F32R, kind="ExternalInput").ap()
    on_d = nc.dram_tensor("on", [128, KT * HPC], F32R, kind="ExternalInput").ap()
    ot_d = nc.dram_tensor("ot", [D, T], F32, kind="ExternalOutput").ap()

    with tile.TileContext(nc) as tc:
        with tc.tile_pool(name="persist", bufs=1) as persist:
            wq_t = persist.tile([128, D8, DG], F32R)
            wk_t = persist.tile([128, D8, DG], F32R)
            wv_t = persist.tile([128, D8, DG], F32R)
            wo_t = persist.tile([128, DG // 128, D], F32R)
            mk_t = persist.tile([128, 128], F32R)
            # head-pair stacked Q.T / K.T: partitions 0-63 head 2p, 64-127 head 2p+1
            qt_t = [persist.tile([128, T], F32R, tag=f"qt{p}", name=f"qt{p}") for p in range(2)]
            kt_t = [persist.tile([128, T], F32R, tag=f"kt{p}", name=f"kt{p}") for p in range(2)]
            # V in natural layout per (key-tile, head): 64 cols + ones col
            # padded to 128 cols: M=65 matmuls fall off the fp32r fast
            # path (measured 429ns vs 227ns); cols 65-127 are junk rows
            # of ctx psum that nothing reads.
            v_t = persist.tile([128, KT, HPC, 128], F32R)
            # normalized ctx.T, stacked like qt (kk=0: heads 0,1; kk=1: heads 2,3)
            ct_t = [persist.tile([128, T], F32R, tag=f"ct{p}", name=f"ct{p}") for p in range(2)]
            ones1 = persist.tile([1, HD], F32R)
            # denominator bank: engine partition bases must be multiples
            # of 32 (and <=64 for matmul operands) -> pack the 16
            # (pair, chunk, head) rows at partitions {0,32,64} x 6 slots
            sums_t = persist.tile([65, 6, NQ], F32R)
            ones3 = persist.tile([65, HD], F32R)

            # wq first (feeds the HAM warm-up matmuls), then zt split fine
            # to engage many DMA queues, then the rest.
            nc.sync.dma_start(wq_t[:], wq_d.rearrange("(a p) c -> p a c", p=128))
            with tc.tile_pool(name="ztp", bufs=1) as ztp:
                zt_t = ztp.tile([128, D8, T], F32R)
                for k8 in range(D8):
                    for half in range(2):
                        nc.sync.dma_start(
                            zt_t[:, k8, half * (T // 2) : (half + 1) * (T // 2)],
                            zt_d[
                                k8 * 128 : (k8 + 1) * 128,
                                half * (T // 2) : (half + 1) * (T // 2),
                            ],
                        )
                nc.sync.dma_start(wk_t[:], wk_d.rearrange("(a p) c -> p a c", p=128))
                nc.sync.dma_start(wv_t[:], wv_d.rearrange("(a p) c -> p a c", p=128))
                nc.sync.dma_start(mk_t[:], mk_d[:])
                # ones: memset can't write f32r on this ISA -> DMA from DRAM
                nc.sync.dma_start(
                    v_t[:, :, :, HD],
                    on_d.rearrange("p (a b) -> p a b", a=KT),
                )
                nc.sync.dma_start(ones1[:], on_d[0:1, 0:HD])
                nc.sync.dma_start(ones3[:], on_d[0:65, 0:HD])
                nc.sync.dma_start(wo_t[:], wo_d.rearrange("(a p) c -> p a c", p=128))

                with tc.tile_pool(name="ps_proj", bufs=2, space="PSUM") as ps_proj:
                    # HAM warm-up: keep PE busy on wq while zt streams in.
                    warm = ps_proj.tile([128, NQ], F32, tag="proj", name="warm")
                    for i in range(96):
                        nc.tensor.matmul(
                            warm[:, 0:DG],
                            wq_t[:, i % D8, 0:128],
                            wq_t[:, (i + 3) % D8, :],
                            start=True,
                            stop=True,
                        )

                    # ── Q.T and K.T projections: out [qkv-col, token] ──
                    for w_t, dst in ((wq_t, qt_t), (wk_t, kt_t)):
                        for m in range(2):
                            for c in range(NCH):
                                ps = ps_proj.tile([128, NQ], F32, tag="proj", name="proj_ps")
                                for k8 in range(D8):
                                    nc.tensor.matmul(
                                        ps[:],
                                        w_t[:, k8, m * 128 : (m + 1) * 128],
                                        zt_t[:, k8, c * NQ : (c + 1) * NQ],
                                        start=(k8 == 0),
                                        stop=(k8 == D8 - 1),
                                    )
                                nc.scalar.copy(
                                    dst[m][:, c * NQ : (c + 1) * NQ], ps[:]
                                )

                    # ── V projection: out [token, qkv-col] ──
                    for vm in range(KT):
                        ps = ps_proj.tile([128, NQ], F32, tag="proj", name="proj_ps")
                        for k8 in range(D8):
                            nc.tensor.matmul(
                                ps[:, 0:DG],
                                zt_t[:, k8, vm * 128 : (vm + 1) * 128],
                                wv_t[:, k8, :],
                                start=(k8 == 0),
                                stop=(k8 == D8 - 1),
                            )
                        nc.scalar.copy(
                            v_t[:, vm, :, 0:HD],
                            ps[:, 0:DG].rearrange("p (h d) -> p h d", h=HPC),
                        )

            # ── attention: chunk-outer / pair-inner; outproj of chunk c-1
            # interleaved into chunk c to keep PE fed through the normalize
            # dependency chain ──
            with (
                tc.tile_pool(name="pbuf", bufs=4) as pbuf,
                tc.tile_pool(name="nrm", bufs=2) as nrm,
                tc.tile_pool(name="stg", bufs=3) as stg,
                tc.tile_pool(name="ps_scores", bufs=3, space="PSUM") as ps_scores,
                tc.tile_pool(name="ps_ctx", bufs=2, space="PSUM") as ps_ctx,
            ):
                # combo idx (p, h) -> (partition row, free slot) for denom bank
                CPOS = [(0, 0), (32, 0), (64, 0), (0, 1)]

                def emit_outproj(c):
                    for mo in range(D // 128):
                        o_ps = ps_scores.tile([128, NQ], F32, tag="s", name="o_ps")
                        for kk in range(2):
                            nc.tensor.matmul(
                                o_ps[:],
                                wo_t[:, kk, mo * 128 : (mo + 1) * 128],
                                ct_t[kk][:, c * NQ : (c + 1) * NQ],
                                start=(kk == 0),
                                stop=(kk == 1),
                            )
                        st = stg.tile([128, NQ], F32, tag="st", name="st")
                        nc.vector.tensor_copy(st[:], o_ps[:])
                        nc.sync.dma_start(
                            ot_d[mo * 128 : (mo + 1) * 128, c * NQ : (c + 1) * NQ],
                            st[:],
                        )

                for c in range(NCH):
                    sums_c = nrm.tile([65, 2, NQ], F32R, tag="sums", name="sums_c")
                    for p in range(2):
                        nbatch = (4 * c + 4) // 2
                        ctxs = [
                            ps_ctx.tile([128, NQ], F32, tag="ctx", name="ctx")
                            for _ in range(2)
                        ]
                        p_tiles = {}

                        def emit_scores(b, h, p=p, c=c, p_tiles=p_tiles):
                            hb = 64 * h
                            s_ps = ps_scores.tile(
                                [128, 2, NQ], F32, tag="s", name="s_ps"
                            )
                            los = []
                            for j in range(2):
                                kt = 2 * b + j
                                d = kt - 4 * c
                                # ragged causal skip: columns < 128*d of a
                                # diagonal tile are entirely masked
                                lo = 128 * d if d > 0 else 0
                                los.append(lo)
                                nc.tensor.matmul(
                                    s_ps[:, j, lo:],
                                    kt_t[p][
                                        hb : hb + 64, kt * 128 : (kt + 1) * 128
                                    ],
                                    qt_t[p][
                                        hb : hb + 64, c * NQ + lo : (c + 1) * NQ
                                    ],
                                    start=True,
                                    stop=True,
                                )
                            p_t = pbuf.tile([128, 2, NQ], F32R, tag="p", name="p_t")
                            if los[0] == los[1]:
                                nc.scalar.activation(
                                    out=p_t[:, :, los[0] :],
                                    in_=s_ps[:, :, los[0] :],
                                    func=EXP, scale=float(SCALE),
                                )
                            else:
                                for j in range(2):
                                    nc.scalar.activation(
                                        out=p_t[:, j, los[j] :],
                                        in_=s_ps[:, j, los[j] :],
                                        func=EXP, scale=float(SCALE),
                                    )
                            for j in range(2):
                                kt = 2 * b + j
                                d = kt - 4 * c
                                if d >= 0:
                                    band = p_t[:, j, 128 * d : 128 * (d + 1)]
                                    nc.vector.tensor_mul(band, band, mk_t[:, :])
                            p_tiles[(b, h)] = (p_t, tuple(los))

                        def emit_pv(b, h, p=p, c=c, p_tiles=p_tiles, ctxs=ctxs):
                            p_t, los = p_tiles.pop((b, h))
                            for j in range(2):
                                kt = 2 * b + j
                                nc.tensor.matmul(
                                    ctxs[h][:, los[j] :],
                                    v_t[:, kt, 2 * p + h, :],
                                    p_t[:, j, los[j] :],
                                    start=(kt == 0),
                                    stop=(kt == 4 * c + 3),
                                )

                        for h in range(2):
                            emit_scores(0, h)
                        if nbatch > 1:
                            for h in range(2):
                                emit_scores(1, h)
                        for b in range(nbatch):
                            if b + 2 < nbatch:
                                for h in range(2):
                                    emit_scores(b + 2, h)
                            for h in range(2):
                                emit_pv(b, h)
                        # evacuate unnormalized ctx + denominator row
                        for h in range(2):
                            prow, slot = CPOS[2 * p + h]
                            nc.vector.tensor_copy(
                                ct_t[p][
                                    h * 64 : h * 64 + HD, c * NQ : (c + 1) * NQ
                                ],
                                ctxs[h][0:HD, :],
                            )
                            nc.vector.tensor_copy(
                                sums_c[prow : prow + 1, slot, :],
                                ctxs[h][HD : HD + 1, :],
                            )
                        if p == 0 and c > 0:
                            emit_outproj(c - 1)

                    # per-chunk normalization: recip = exp(-ln(s)), broadcast
                    # along partitions via K=1 ones-matmul, scale ct in place
                    nc.scalar.activation(out=sums_c[:], in_=sums_c[:], func=LN)
                    nc.scalar.activation(
                        out=sums_c[:], in_=sums_c[:], func=EXP, scale=-1.0
                    )
                    for p in range(2):
                        for h in range(2):
                            prow, slot = CPOS[2 * p + h]
                            bc_ps = ps_scores.tile(
                                [128, NQ], F32, tag="s", name="bc_ps"
                            )
                            nc.tensor.matmul(
                                bc_ps[0:HD, :],
                                ones3[prow : prow + 1, :],
                                sums_c[prow : prow + 1, slot, :],
                                start=True,
                                stop=True,
                            )
                            ct_slice = ct_t[p][
                                h * 64 : h * 64 + HD, c * NQ : (c + 1) * NQ
                            ]
                            nc.vector.tensor_mul(
                                ct_slice, ct_slice, bc_ps[0:HD, :]
                            )

                emit_outproj(NCH - 1)

    return nc


def _host_inputs(z, w_q, w_k, w_v, w_o):
    """Per-core input maps (host-side sharding + transposes)."""
    z = np.asarray(z, dtype=np.float32)
    w_q = np.asarray(w_q, dtype=np.float32)
    w_k = np.asarray(w_k, dtype=np.float32)
    w_v = np.asarray(w_v, dtype=np.float32)
    w_o = np.asarray(w_o, dtype=np.float32)

    pj = np.arange(128)[:, None]
    fi = np.arange(128)[None, :]
    tri = (fi >= pj).astype(np.float32)  # [128, 128] causal boundary band

    zt = [np.ascontiguousarray(z[b].T) for b in range(B)]
    in_maps = []
    for core in range(8):
        b, g = core // 4, core % 4
        cs = slice(g * DG, (g + 1) * DG)
        in_maps.append(
            {
                "zt": zt[b],
                "wq": np.ascontiguousarray(w_q[:, cs]),
                "wk": np.ascontiguousarray(w_k[:, cs]),
                "wv": np.ascontiguousarray(w_v[:, cs]),
                "wo": np.ascontiguousarray(w_o[cs, :]),
                "mk": tri,
                "on": np.ones((128, KT * HPC), dtype=np.float32),
            }
        )
    return in_maps


def run(z, w_q, w_k, w_v, w_o, trace=False, trace_cores=None):
    """Build + run on 8 cores; returns (output [B,T,D], BassKernelResults)."""
    nc = build_kernel()
    n = _split_excess_waits(nc)
    if n:
        print(f"[kernel] split {n} excess sync-waits onto nops", file=sys.stderr)
    in_maps = _host_inputs(z, w_q, w_k, w_v, w_o)
    res = bass_utils.run_bass_kernel_spmd(
        nc, in_maps, list(range(8)), trace=trace, trace_cores=trace_cores
    )
    out = np.zeros((B, T, D), dtype=np.float64)
    for core in range(8):
        out[core // 4] += res.results[core]["ot"].T.astype(np.float64)
    return out.astype(np.float32), res


def kernel(z, w_q, w_k, w_v, w_o):
    out, _ = run(z, w_q, w_k, w_v, w_o, trace=False)
    return out


# revision 19
# speedup vs baseline: 1.0490x; 1.0490x over previous
"""Causal multi-head self-attention on 8 TRN2 NeuronCores (Bass/Tile).

Problem: z[B=2,T=2048,D=1024], per-head dim 64, H=16 heads, fp32.
Sharding: core = b*4 + g  (b = batch, g = head-group of 4 heads).
Each core computes, for its batch b and heads 4g..4g+3:
    Q.T/K.T = (Wq/Wk slice).T-projection of z.T   [256, 2048] (head-stacked)
    V       = z @ Wv slice                         [2048, 256] (+ ones column)
    S.T     = K.T-slices vs Q.T  (partition = key j, free = query i)
    P       = exp(S/8) * causal mask   (no max-subtraction needed: |S|≲8σ)
    ctx.T   = [V | 1].T @ P   -> row 64 holds the softmax denominators
    out.T  += Wo-rows.T @ (ctx.T / denom)          [1024, 2048] partial
Host sums the 4 per-batch partials and transposes back.

All matmuls run as float32r (full-rate fp32 path; measured ~1.4e-4 relmax
vs fp64 on this HW, identical to plain fp32 output in the probe).
"""
import sys
import types

import numpy as np

# ── antenv.axon_hooks shim (NTFF profiling; agent image lacks the module) ──
import antenv  # noqa: F401

if "antenv.axon_hooks" not in sys.modules:
    _hooks = types.ModuleType("antenv.axon_hooks")
    _HOOK = [None]
    _hooks.set_axon_ntff_profile_hook = lambda h: _HOOK.__setitem__(0, h)
    _hooks.get_axon_ntff_profile_hook = lambda: _HOOK[0]
    sys.modules["antenv.axon_hooks"] = _hooks
    antenv.axon_hooks = _hooks
    try:
        from trn_agent_boot.trn_boot import _ntff_profile_via_ctypes

        _hooks.set_axon_ntff_profile_hook(
            _ntff_profile_via_ctypes("/opt/axon/libaxon_pjrt.so")
        )
    except Exception:
        pass

import concourse.bass as bass  # noqa: E402
import concourse.tile as tile  # noqa: E402
import concourse.mybir as mybir  # noqa: E402
import concourse.bass_utils as bass_utils  # noqa: E402
from bass_rust import ScopedClock  # noqa: E402

bass_utils.upload_artifacts = lambda tmpdir: ""

F32 = mybir.dt.float32
F32R = mybir.dt.float32r
EXP = mybir.ActivationFunctionType.Exp
LN = mybir.ActivationFunctionType.Ln

# ── workaround: this walrus build allows max ONE sync-wait per instruction ──
_wsplit = [0]


def _split_excess_waits(nc, limit=1):
    n = 0
    for fn in nc.m.functions:
        for blk in fn.blocks:
            out = []
            for inst in blk.instructions:
                si = inst.sync_info
                if si is not None and len(si.on_wait) > limit:
                    ws = list(si.on_wait)
                    keep = ws[-limit:]
                    hoist = ws[:-limit]
                    for i in range(0, len(hoist), limit):
                        _wsplit[0] += 1
                        out.append(
                            mybir.InstNoOp(
                                name=f"I-wsplit-{_wsplit[0]}",
                                engine=inst.engine,
                                sync_info=mybir.SyncInfo(
                                    on_wait=hoist[i : i + limit], on_update=[]
                                ),
                                bass_nofuse=True,
                            )
                        )
                        n += 1
                    si.on_wait = keep
                out.append(inst)
            blk.instructions = out
    return n


def _patched_drain_and_barrier(self, tick_clock, wait_clock):
    nc = self.nc
    drain_inst = nc.sync.drain()
    wait_clock.add_sem_waits(
        drain_inst.ins, ScopedClock({None: tick_clock.global_clock})
    )
    si = drain_inst.ins.sync_info
    if si is not None and len(si.on_wait) > 1:
        waits = list(si.on_wait)
        si.on_wait = waits[:1]
        for w in waits[1:]:
            d2 = nc.sync.drain()
            d2.ins.sync_info = mybir.SyncInfo(on_wait=[w], on_update=[])
    nc.all_engine_barrier()
    assert self.sems is not None
    popped = nc._tile_sem_poison_stack.pop()
    assert popped is self._sem_poison
    nc.clear_and_free_semaphores(list(self.sems.allocated().values()))
    nc.all_engine_barrier()


tile.TileContext._drain_and_barrier = _patched_drain_and_barrier

# ── problem shape (hardcoded) ──
B, T, D, H, HD = 2, 2048, 1024, 16, 64
HPC = 4  # heads per core
DG = HPC * HD  # 256 projection cols per core
NQ = 512  # query-chunk width (one PSUM bank of fp32)
KT = T // 128  # 16 key tiles
NCH = T // NQ  # 4 query chunks
D8 = D // 128  # 8 contraction tiles
SCALE = 1.0 / np.sqrt(HD)


def build_kernel():
    nc = bass.Bass("TRN2", target_bir_lowering=False, debug=False)
    zt_d = nc.dram_tensor("zt", [D, T], F32R, kind="ExternalInput").ap()
    wq_d = nc.dram_tensor("wq", [D, DG], F32R, kind="ExternalInput").ap()
    wk_d = nc.dram_tensor("wk", [D, DG], F32R, kind="ExternalInput").ap()
    wv_d = nc.dram_tensor("wv", [D, DG], F32R, kind="ExternalInput").ap()
    wo_d = nc.dram_tensor("wo", [DG, D], F32R, kind="ExternalInput").ap()
    mk_d = nc.dram_tensor("mk", [128, 128], F32R, kind="ExternalInput").ap()
    on_d = nc.dram_tensor("on", [128, KT * HPC], F32R, kind="ExternalInput").ap()
    ot_d = nc.dram_tensor("ot", [D, T], F32, kind="ExternalOutput").ap()

    with tile.TileContext(nc) as tc:
        with tc.tile_pool(name="persist", bufs=1) as persist:
            wq_t = persist.tile([128, D8, DG], F32R)
            wk_t = persist.tile([128, D8, DG], F32R)
            wv_t = persist.tile([128, D8, DG], F32R)
            wo_t = persist.tile([128, DG // 128, D], F32R)
            mk_t = persist.tile([128, 128], F32R)
            # head-pair stacked Q.T / K.T: partitions 0-63 head 2p, 64-127 head 2p+1
            qt_t = [persist.tile([128, T], F32R, tag=f"qt{p}", name=f"qt{p}") for p in range(2)]
            kt_t = [persist.tile([128, T], F32R, tag=f"kt{p}", name=f"kt{p}") for p in range(2)]
            # V in natural layout per (key-tile, head): 64 cols + ones col
            # padded to 128 cols: M=65 matmuls fall off the fp32r fast
            # path (measured 429ns vs 227ns); cols 65-127 are junk rows
            # of ctx psum that nothing reads.
            v_t = persist.tile([128, KT, HPC, 128], F32R)
            # normalized ctx.T, stacked like qt (kk=0: heads 0,1; kk=1: heads 2,3)
            ct_t = [persist.tile([128, T], F32R, tag=f"ct{p}", name=f"ct{p}") for p in range(2)]
            ones1 = persist.tile([1, HD], F32R)
            # denominator bank: engine partition bases must be multiples
            # of 32 (and <=64 for matmul operands) -> pack the 16
            # (pair, chunk, head) rows at partitions {0,32,64} x 6 slots
            sums_t = persist.tile([65, 6, NQ], F32R)
            ones3 = persist.tile([65, HD], F32R)

            # wq first (feeds the HAM warm-up matmuls), then zt split fine
            # to engage many DMA queues, then the rest.
            nc.sync.dma_start(wq_t[:], wq_d.rearrange("(a p) c -> p a c", p=128))
            with tc.tile_pool(name="ztp", bufs=1) as ztp:
                zt_t = ztp.tile([128, D8, T], F32R)
                for k8 in range(D8):
                    for half in range(2):
                        nc.sync.dma_start(
                            zt_t[:, k8, half * (T // 2) : (half + 1) * (T // 2)],
                            zt_d[
                                k8 * 128 : (k8 + 1) * 128,
                                half * (T // 2) : (half + 1) * (T // 2),
                            ],
                        )
                nc.sync.dma_start(wk_t[:], wk_d.rearrange("(a p) c -> p a c", p=128))
                nc.sync.dma_start(wv_t[:], wv_d.rearrange("(a p) c -> p a c", p=128))
                nc.sync.dma_start(mk_t[:], mk_d[:])
                # ones: memset can't write f32r on this ISA -> DMA from DRAM
                nc.sync.dma_start(
                    v_t[:, :, :, HD],
                    on_d.rearrange("p (a b) -> p a b", a=KT),
                )
                nc.sync.dma_start(ones1[:], on_d[0:1, 0:HD])
                nc.sync.dma_start(ones3[:], on_d[0:65, 0:HD])
                nc.sync.dma_start(wo_t[:], wo_d.rearrange("(a p) c -> p a c", p=128))

                with tc.tile_pool(name="ps_proj", bufs=2, space="PSUM") as ps_proj:
                    # HAM warm-up: keep PE busy on wq while zt streams in.
                    warm = ps_proj.tile([128, NQ], F32, tag="proj", name="warm")
                    for i in range(96):
                        nc.tensor.matmul(
                            warm[:, 0:DG],
                            wq_t[:, i % D8, 0:128],
                            wq_t[:, (i + 3) % D8, :],
                            start=True,
                            stop=True,
                        )

                    # ── Q.T and K.T projections: out [qkv-col, token] ──
                    for w_t, dst in ((wq_t, qt_t), (wk_t, kt_t)):
                        for m in range(2):
                            for c in range(NCH):
                                ps = ps_proj.tile([128, NQ], F32, tag="proj", name="proj_ps")
                                for k8 in range(D8):
                                    nc.tensor.matmul(
                                        ps[:],
                                        w_t[:, k8, m * 128 : (m + 1) * 128],
                                        zt_t[:, k8, c * NQ : (c + 1) * NQ],
                                        start=(k8 == 0),
                                        stop=(k8 == D8 - 1),
                                    )
                                nc.scalar.copy(
                                    dst[m][:, c * NQ : (c + 1) * NQ], ps[:]
                                )

                    # ── V projection: out [token, qkv-col] ──
                    for vm in range(KT):
                        ps = ps_proj.tile([128, NQ], F32, tag="proj", name="proj_ps")
                        for k8 in range(D8):
                            nc.tensor.matmul(
                                ps[:, 0:DG],
                                zt_t[:, k8, vm * 128 : (vm + 1) * 128],
                                wv_t[:, k8, :],
                                start=(k8 == 0),
                                stop=(k8 == D8 - 1),
                            )
                        nc.scalar.copy(
                            v_t[:, vm, :, 0:HD],
                            ps[:, 0:DG].rearrange("p (h d) -> p h d", h=HPC),
                        )

            # ── attention: chunk-outer / pair-inner; outproj of chunk c-1
            # interleaved into chunk c to keep PE fed through the normalize
            # dependency chain ──
            with (
                tc.tile_pool(name="pbuf", bufs=4) as pbuf,
                tc.tile_pool(name="nrm", bufs=2) as nrm,
                tc.tile_pool(name="stg", bufs=3) as stg,
                tc.tile_pool(name="ps_scores", bufs=3, space="PSUM") as ps_scores,
                tc.tile_pool(name="ps_ctx", bufs=2, space="PSUM") as ps_ctx,
            ):
                # combo idx (p, h) -> (partition row, free slot) for denom bank
                CPOS = [(0, 0), (32, 0), (64, 0), (0, 1)]

                def emit_recip(c, sums_c):
                    # recip = exp(-ln(s)) in place, batched over the chunk's
                    # 4 (pair, head) denominator rows
                    nc.scalar.activation(out=sums_c[:], in_=sums_c[:], func=LN)
                    nc.scalar.activation(
                        out=sums_c[:], in_=sums_c[:], func=EXP, scale=-1.0
                    )

                def emit_normalize(c, sums_c):
                    # partition-broadcast each recip row (K=1 ones-matmul),
                    # then scale ct in place
                    for p in range(2):
                        for h in range(2):
                            prow, slot = CPOS[2 * p + h]
                            bc_ps = ps_scores.tile(
                                [128, NQ], F32, tag="s", name="bc_ps"
                            )
                            nc.tensor.matmul(
                                bc_ps[0:HD, :],
                                ones3[prow : prow + 1, :],
                                sums_c[prow : prow + 1, slot, :],
                                start=True,
                                stop=True,
                            )
                            ct_slice = ct_t[p][
                                h * 64 : h * 64 + HD, c * NQ : (c + 1) * NQ
                            ]
                            nc.vector.tensor_mul(
                                ct_slice, ct_slice, bc_ps[0:HD, :]
                            )

                sums_tiles = {}

                def emit_outproj(c):
                    for mo in range(D // 128):
                        o_ps = ps_scores.tile([128, NQ], F32, tag="s", name="o_ps")
                        for kk in range(2):
                            nc.tensor.matmul(
                                o_ps[:],
                                wo_t[:, kk, mo * 128 : (mo + 1) * 128],
                                ct_t[kk][:, c * NQ : (c + 1) * NQ],
                                start=(kk == 0),
                                stop=(kk == 1),
                            )
                        st = stg.tile([128, NQ], F32, tag="st", name="st")
                        nc.vector.tensor_copy(st[:], o_ps[:])
                        nc.sync.dma_start(
                            ot_d[mo * 128 : (mo + 1) * 128, c * NQ : (c + 1) * NQ],
                            st[:],
                        )

                for c in range(NCH):
                    sums_c = nrm.tile([65, 2, NQ], F32R, tag="sums", name="sums_c")
                    sums_tiles[c] = sums_c
                    if c > 0:
                        emit_recip(c - 1, sums_tiles[c - 1])
                    for p in range(2):
                        nbatch = (4 * c + 4) // 2
                        ctxs = [
                            ps_ctx.tile([128, NQ], F32, tag="ctx", name="ctx")
                            for _ in range(2)
                        ]
                        p_tiles = {}

                        def emit_scores(b, h, p=p, c=c, p_tiles=p_tiles):
                            hb = 64 * h
                            s_ps = ps_scores.tile(
                                [128, 2, NQ], F32, tag="s", name="s_ps"
                            )
                            los = []
                            for j in range(2):
                                kt = 2 * b + j
                                d = kt - 4 * c
                                # ragged causal skip: columns < 128*d of a
                                # diagonal tile are entirely masked
                                lo = 128 * d if d > 0 else 0
                                los.append(lo)
                                nc.tensor.matmul(
                                    s_ps[:, j, lo:],
                                    kt_t[p][
                                        hb : hb + 64, kt * 128 : (kt + 1) * 128
                                    ],
                                    qt_t[p][
                                        hb : hb + 64, c * NQ + lo : (c + 1) * NQ
                                    ],
                                    start=True,
                                    stop=True,
                                )
                            p_t = pbuf.tile([128, 2, NQ], F32R, tag="p", name="p_t")
                            if los[0] == los[1]:
                                nc.scalar.activation(
                                    out=p_t[:, :, los[0] :],
                                    in_=s_ps[:, :, los[0] :],
                                    func=EXP, scale=float(SCALE),
                                )
                            else:
                                for j in range(2):
                                    nc.scalar.activation(
                                        out=p_t[:, j, los[j] :],
                                        in_=s_ps[:, j, los[j] :],
                                        func=EXP, scale=float(SCALE),
                                    )
                            for j in range(2):
                                kt = 2 * b + j
                                d = kt - 4 * c
                                if d >= 0:
                                    band = p_t[:, j, 128 * d : 128 * (d + 1)]
                                    nc.vector.tensor_mul(band, band, mk_t[:, :])
                            p_tiles[(b, h)] = (p_t, tuple(los))

                        def emit_pv(b, h, p=p, c=c, p_tiles=p_tiles, ctxs=ctxs):
                            p_t, los = p_tiles.pop((b, h))
                            for j in range(2):
                                kt = 2 * b + j
                                nc.tensor.matmul(
                                    ctxs[h][:, los[j] :],
                                    v_t[:, kt, 2 * p + h, :],
                                    p_t[:, j, los[j] :],
                                    start=(kt == 0),
                                    stop=(kt == 4 * c + 3),
                                )

                        for h in range(2):
                            emit_scores(0, h)
                        if nbatch > 1:
                            for h in range(2):
                                emit_scores(1, h)
                        for b in range(nbatch):
                            if b + 2 < nbatch:
                                for h in range(2):
                                    emit_scores(b + 2, h)
                            for h in range(2):
                                emit_pv(b, h)
                        # evacuate unnormalized ctx + denominator row
                        for h in range(2):
                            prow, slot = CPOS[2 * p + h]
                            nc.vector.tensor_copy(
                                ct_t[p][
                                    h * 64 : h * 64 + HD, c * NQ : (c + 1) * NQ
                                ],
                                ctxs[h][0:HD, :],
                            )
                            nc.vector.tensor_copy(
                                sums_c[prow : prow + 1, slot, :],
                                ctxs[h][HD : HD + 1, :],
                            )
                        if p == 0 and c > 0:
                            emit_normalize(c - 1, sums_tiles[c - 1])
                        if p == 1 and c > 0:
                            emit_outproj(c - 1)

                emit_recip(NCH - 1, sums_tiles[NCH - 1])
                emit_normalize(NCH - 1, sums_tiles[NCH - 1])
                emit_outproj(NCH - 1)

    return nc


def _host_inputs(z, w_q, w_k, w_v, w_o):
    """Per-core input maps (host-side sharding + transposes)."""
    z = np.asarray(z, dtype=np.float32)
    w_q = np.asarray(w_q, dtype=np.float32)
    w_k = np.asarray(w_k, dtype=np.float32)
    w_v = np.asarray(w_v, dtype=np.float32)
    w_o = np.asarray(w_o, dtype=np.float32)

    pj = np.arange(128)[:, None]
    fi = np.arange(128)[None, :]
    tri = (fi >= pj).astype(np.float32)  # [128, 128] causal boundary band

    zt = [np.ascontiguousarray(z[b].T) for b in range(B)]
    in_maps = []
    for core in range(8):
        b, g = core // 4, core % 4
        cs = slice(g * DG, (g + 1) * DG)
        in_maps.append(
            {
                "zt": zt[b],
                "wq": np.ascontiguousarray(w_q[:, cs]),
                "wk": np.ascontiguousarray(w_k[:, cs]),
                "wv": np.ascontiguousarray(w_v[:, cs]),
                "wo": np.ascontiguousarray(w_o[cs, :]),
                "mk": tri,
                "on": np.ones((128, KT * HPC), dtype=np.float32),
            }
        )
    return in_maps


def run(z, w_q, w_k, w_v, w_o, trace=False, trace_cores=None):
    """Build + run on 8 cores; returns (output [B,T,D], BassKernelResults)."""
    nc = build_kernel()
    n = _split_excess_waits(nc)
    if n:
        print(f"[kernel] split {n} excess sync-waits onto nops", file=sys.stderr)
    in_maps = _host_inputs(z, w_q, w_k, w_v, w_o)
    res = bass_utils.run_bass_kernel_spmd(
        nc, in_maps, list(range(8)), trace=trace, trace_cores=trace_cores
    )
    out = np.zeros((B, T, D), dtype=np.float64)
    for core in range(8):
        out[core // 4] += res.results[core]["ot"].T.astype(np.float64)
    return out.astype(np.float32), res


def kernel(z, w_q, w_k, w_v, w_o):
    out, _ = run(z, w_q, w_k, w_v, w_o, trace=False)
    return out
